# revision 25
# baseline (speedup 1.0000x reference)
"""Multi-head attention kernel for 8 TRN2 NeuronCores.

Problem: B=2, S=2048, D=1024, H=16 heads, head_dim=64, fp32 I/O.

Sharding (per the tensor-parallel hint): 8 cores = 2 batches x 4 head-groups.
Core c handles batch c//4 and heads [4*(c%4), 4*(c%4)+4). Each core:
  - projects its head-slice qT/kT (feature-on-partition layout, 2 heads per
    128-partition tile) and v (natural layout, with an appended ones column),
  - computes scoresT = k @ q.T per head with K=64 row-tiled matmuls (two heads
    run CONCURRENTLY in the PE array: row_grp 0-1 vs 2-3),
  - exp on ScalarE with the 1/sqrt(64) scale and the -1e9 mask folded into the
    activation's scale/bias,
  - attn@v with the [v|1] trick: the ones column makes the softmax denominator
    fall out of the same matmul stream (PSUM row 64),
  - normalizes via reciprocal + a rank-1 PE broadcast matmul,
  - computes a partial output projection over its 256 features.
Host sums the 4 partials per batch and adds the output bias.
All matmul operands are bf16 (fp32 matmul is 4x slower on the PE array;
fp8 was measured numerically and blows the 2e-2 error budget);
accumulation is fp32 in PSUM and the returned partials are fp32.

v2 changes vs the 236us baseline:
  - xT arrives as host-packed xP with all 8 d-chunks of an s-chunk contiguous
    per partition: ONE DMA per s-chunk (128 descriptors x 8KB) instead of 8.
    SWDGE descriptor-gen was ~1.6us per DMA; startup drops ~14us -> ~5us.
  - v_proj computes all 4 heads per s-row chunk in one psum (N=256): the
    N=128 variant was LDWEIGHTS-bound (97ns load vs 53ns stream), this one
    is balanced. ~25us of effective PE -> ~14us, and half the v drip slots.
  - the FINAL block's normalize uses a direct DVE reciprocal on the (1,512)
    Z rows instead of the ~4us GpSimd DMA reshape bounce (which is hidden
    mid-stream but exposed at the tail): kills the 5.4us tail PE gap.
"""

import numpy as np
import ml_dtypes

import concourse.mybir as mybir
import concourse.tile as tile
from concourse import bacc
from concourse.bass_utils import run_bass_kernel_spmd

BF16 = mybir.dt.bfloat16
FP32 = mybir.dt.float32

B, S, D = 2, 2048, 1024
NH, DH = 16, 64
NCORES = 8
GROUPS = 4                 # head-groups (cores per batch)
HL = NH // GROUPS          # heads per core = 4
FL = HL * DH               # features per core = 256
NPAIR = HL // 2            # head pairs per core = 2

SC = 512                   # i/s chunk (PSUM bank = 512 fp32)
JC = 128                   # j chunk (partition dim)
DCH = D // 128             # contraction chunks over embed dim = 8
N_SC = S // SC             # 4
N_JC = S // JC             # 16


def build_kernel():
    nc = bacc.Bacc("TRN2", target_bir_lowering=False, debug=False)

    # xP: s-chunk-major packed xT: [p, sc*(8*512) + dc*512 + c] =
    # xT[dc*128+p, sc*512+c]. Both src and dst are contiguous 8KB per
    # partition per s-chunk -> one 128-descriptor DMA per s-chunk.
    xP = nc.dram_tensor("xP", [128, N_SC * DCH * SC], BF16, kind="ExternalInput")
    # weights arrive host-packed as one contiguous (128, 2048) block each so a
    # single large DMA loads them (24 small DMAs serialized the Sync queue)
    wq = nc.dram_tensor("wq", [128, DCH * FL], BF16, kind="ExternalInput")
    wk = nc.dram_tensor("wk", [128, DCH * FL], BF16, kind="ExternalInput")
    wv = nc.dram_tensor("wv", [128, DCH * FL], BF16, kind="ExternalInput")
    wo = nc.dram_tensor("wo", [128, 2 * D], BF16, kind="ExternalInput")
    bqc = nc.dram_tensor("bqc", [128, 2], FP32, kind="ExternalInput")
    bkc = nc.dram_tensor("bkc", [128, 2], FP32, kind="ExternalInput")
    bvb = nc.dram_tensor("bvb", [128, FL], FP32, kind="ExternalInput")
    mb = nc.dram_tensor("mb", [128, N_JC], FP32, kind="ExternalInput")
    # partials are summed on the host anyway; bf16 halves the store traffic
    # and costs ~1e-3 extra relative error.
    out = nc.dram_tensor("out", [S, D], BF16, kind="ExternalOutput")

    with tile.TileContext(nc) as tc:
        with (
            tc.tile_pool(name="weights", bufs=1) as wpool,
            tc.tile_pool(name="acts", bufs=1) as apool,
            tc.tile_pool(name="exps", bufs=20) as epool,
            tc.tile_pool(name="stages", bufs=6) as spool,
            tc.tile_pool(name="smalls", bufs=6) as smpool,
            tc.tile_pool(name="scores", bufs=2, space="PSUM") as scpool,
            tc.tile_pool(name="attnout", bufs=2, space="PSUM") as aopool,
            tc.tile_pool(name="projacc", bufs=2, space="PSUM") as prpool,
        ):
            # ---- resident inputs ----
            # xt_all layout is s-chunk-major to match xP: the (dc, sc) slice
            # lives at cols sc*4096 + dc*512.
            xt_all = wpool.tile([128, N_SC * DCH * SC], BF16, name="xt_all")

            def xt(dc, sc):
                base = sc * DCH * SC + dc * SC
                return xt_all[:, base:base + SC]

            def xtj(dc, jc):
                # 128-col j-slice inside the right s-chunk
                sc, off = divmod(jc * JC, SC)
                base = sc * DCH * SC + dc * SC + off
                return xt_all[:, base:base + JC]

            # s-chunk 0 on GpSimd (SWDGE) so the Sync ring starts on weights
            # immediately; later s-chunks ride Sync interleaved between the
            # weight loads in consumption order.
            nc.gpsimd.dma_start(
                out=xt_all[:, 0:DCH * SC],
                in_=xP.ap()[:, 0:DCH * SC],
            )

            def sc_dma(sc):
                nc.sync.dma_start(
                    out=xt_all[:, sc * DCH * SC:(sc + 1) * DCH * SC],
                    in_=xP.ap()[:, sc * DCH * SC:(sc + 1) * DCH * SC],
                )

            # Sync-ring order is consumption-priority order: the first exp
            # needs wk+wq (and sc0 from the GpSimd ring); everything else can
            # trail. The HBM bus is shared by all 8 cores loading at once
            # (~180GB/s effective), so the minimal prefix set must not queue
            # behind bulk.
            wk_sb = wpool.tile([128, DCH * FL], BF16, name="wk_sb")
            nc.sync.dma_start(out=wk_sb, in_=wk.ap())
            wkt = [wk_sb[:, dc * FL:(dc + 1) * FL] for dc in range(DCH)]
            bk_sb = wpool.tile([128, 2], FP32, name="bk_sb")
            nc.sync.dma_start(out=bk_sb, in_=bkc.ap())
            wq_sb = wpool.tile([128, DCH * FL], BF16, name="wq_sb")
            nc.sync.dma_start(out=wq_sb, in_=wq.ap())
            wqt = [wq_sb[:, dc * FL:(dc + 1) * FL] for dc in range(DCH)]
            bq_sb = wpool.tile([128, 2], FP32, name="bq_sb")
            nc.sync.dma_start(out=bq_sb, in_=bqc.ap())
            mb_sb = wpool.tile([128, N_JC], FP32, name="mb_sb")
            nc.sync.dma_start(out=mb_sb, in_=mb.ap())
            wv_sb = wpool.tile([128, DCH * FL], BF16, name="wv_sb")
            nc.sync.dma_start(out=wv_sb, in_=wv.ap())
            wvt = [wv_sb[:, dc * FL:(dc + 1) * FL] for dc in range(DCH)]
            bv_sb = wpool.tile([128, FL], FP32, name="bv_sb")
            nc.sync.dma_start(out=bv_sb, in_=bvb.ap())
            sc_dma(1)
            sc_dma(2)
            sc_dma(3)
            wo_sb = wpool.tile([128, 2 * D], BF16, name="wo_sb")
            nc.sync.dma_start(out=wo_sb, in_=wo.ap())
            wot = [wo_sb[:, fc * D:(fc + 1) * D] for fc in range(2)]

            # ones column at partition 64 for the recip broadcast matmul
            ones65 = wpool.tile([65, 64], BF16, name="ones65")
            nc.vector.memset(ones65[64:65, :], 1.0)
            # warm the ScalarE Exp table set while DMAs stream (saves the
            # ~2.7us ACT_TABLE_LOAD from delaying the first real exp)
            warm = smpool.tile([1, 4], FP32, name="warm", tag="warm")
            nc.vector.memset(warm, 1.0)
            nc.scalar.activation(warm, warm, mybir.ActivationFunctionType.Exp)
            # warm the PE while the input DMAs stream: ~10 junk matmuls ramp
            # the HAM clock gate to 8/8 (full 2.4GHz needs ~3us of continuous
            # PE busy), so the k0/q0/v0 prefix runs at full speed instead of
            # pstate-low (~585ns/MM cold vs ~215 warm). Saves ~10us of
            # startup.
            junk = wpool.tile([128, SC], BF16, name="junk")
            nc.vector.memset(junk, 1.0)
            for _ in range(12):
                wps = prpool.tile([128, SC], FP32, name="wps", tag="ps")
                nc.tensor.matmul(wps, lhsT=junk[:, 0:128], rhs=junk)

            # ---- persistent activations ----
            # qT/kT: tile p holds features [128p,128p+128) = heads 2p,2p+1
            qt = [apool.tile([128, S], BF16, name=f"qt{p}") for p in range(2)]
            kt = [apool.tile([128, S], BF16, name=f"kt{p}") for p in range(2)]
            # v natural: tile sc = rows [128sc,128sc+128), layout (128, 4 heads, 65)
            vt = [apool.tile([128, HL, 65], BF16, name=f"vt{sc}") for sc in range(N_JC)]
            # normalized attention output, transposed: (features, S)
            at = [apool.tile([128, S], BF16, name=f"at{p}") for p in range(2)]

            qk_open = {}  # key -> open psum accumulation tile

            def qk_half(dst, w_tiles, bias_sb, sc, fc, half):
                """Half of a qT/kT projection s-chunk (4 of 8 d-accumulation
                matmuls, ~0.9us of PE) so drip slots stay small. The psum
                group stays open between halves.

                Runs at medium priority: in block 0 the kt/qt chains gate the
                scores (and with them the whole exp stream), while the v
                drips sharing those slots only gate the latency-tolerant
                attn@v accumulation."""
                key = (id(dst), sc)
                with tc.high_priority(offset=5_000_000):
                    if half == 0:
                        ps = prpool.tile([128, SC], FP32, name="ps", tag="ps")
                        qk_open[key] = ps
                    else:
                        ps = qk_open.pop(key)
                    for dc in range(half * 4, half * 4 + 4):
                        nc.tensor.matmul(
                            ps,
                            lhsT=w_tiles[dc][:, fc * 128:(fc + 1) * 128],
                            rhs=xt(dc, sc),
                            start=(dc == 0),
                            stop=(dc == DCH - 1),
                        )
                    if half == 1:
                        nc.vector.tensor_scalar_add(
                            dst[:, sc * SC:(sc + 1) * SC], ps, bias_sb[:, fc:fc + 1]
                        )

            def qk_quarter(dst, w_tiles, bias_sb, sc, fc, quarter):
                """Quarter of a qT/kT projection s-chunk (2 matmuls, ~0.43us)
                for the ScalarE-paced blocks: a 0.85us half in one drip slot
                pushes the next scores pair (and with it the exp) ~0.4us late;
                quarters keep every slot under the exp pace."""
                key = (id(dst), sc)
                if quarter == 0:
                    ps = prpool.tile([128, SC], FP32, name="ps", tag="ps")
                    qk_open[key] = ps
                else:
                    ps = qk_open[key]
                for dc in range(quarter * 2, quarter * 2 + 2):
                    nc.tensor.matmul(
                        ps,
                        lhsT=w_tiles[dc][:, fc * 128:(fc + 1) * 128],
                        rhs=xt(dc, sc),
                        start=(dc == 0),
                        stop=(dc == DCH - 1),
                    )
                if quarter == 3:
                    del qk_open[key]
                    nc.vector.tensor_scalar_add(
                        dst[:, sc * SC:(sc + 1) * SC], ps, bias_sb[:, fc:fc + 1]
                    )

            def qk_full(dst, w_tiles, bias_sb, sc, fc):
                qk_half(dst, w_tiles, bias_sb, sc, fc, 0)
                qk_half(dst, w_tiles, bias_sb, sc, fc, 1)

            def v_proj(sc):
                """v rows [128sc,+128) for ALL 4 heads in one N=256 stream
                (~0.9us of PE; the old per-pair N=128 version was
                LDWEIGHTS-bound at ~1.6us for the same work)."""
                ps = prpool.tile([128, FL], FP32, name="vps", tag="ps")
                for dc in range(DCH):
                    nc.tensor.matmul(
                        ps,
                        lhsT=xtj(dc, sc),
                        rhs=wvt[dc],
                        start=(dc == 0),
                        stop=(dc == DCH - 1),
                    )
                nc.vector.tensor_add(
                    vt[sc][:, :, 0:64],
                    ps.rearrange("p (h d) -> p h d", h=HL),
                    bv_sb.rearrange("p (h d) -> p h d", h=HL),
                )
                nc.vector.memset(vt[sc][:, :, 64:65], 1.0)

            pending_norm = {}

            def scores_exp(pair, ic, jc):
                i_sl = slice(ic * SC, (ic + 1) * SC)
                # high_priority: the Tile scheduler pops READY instructions by
                # emission priority, so a drip emitted in an earlier slot
                # otherwise preempts these matmuls at every PE-idle event,
                # splitting the concurrent pair and pushing the exp (the
                # global pace) ~0.3-0.6us late per occurrence.
                with tc.high_priority(offset=10_000_000):
                    sc_ps = scpool.tile([128, 2 * SC], FP32, name="sc_ps")
                    # scoresT = k @ q.T, two heads row-tiled (K=64 each,
                    # row_grp 0-1 vs 2-3 -> they stream concurrently)
                    nc.tensor.matmul(
                        sc_ps[:, 0:SC],
                        lhsT=kt[pair][0:64, jc * JC:(jc + 1) * JC],
                        rhs=qt[pair][0:64, i_sl],
                    )
                    nc.tensor.matmul(
                        sc_ps[:, SC:2 * SC],
                        lhsT=kt[pair][64:128, jc * JC:(jc + 1) * JC],
                        rhs=qt[pair][64:128, i_sl],
                    )
                    ex = epool.tile([128, 2 * SC], BF16, name="ex")
                    nc.scalar.activation(
                        ex, sc_ps, mybir.ActivationFunctionType.Exp,
                        bias=mb_sb[:, jc:jc + 1], scale=1.0 / np.sqrt(DH),
                    )
                return ex

            def attn_acc(pair, jc, ex, outA, outB):
                nc.tensor.matmul(
                    outA, lhsT=vt[jc][:, 2 * pair, :], rhs=ex[:, 0:SC],
                    start=(jc == 0), stop=(jc == N_JC - 1),
                )
                nc.tensor.matmul(
                    outB, lhsT=vt[jc][:, 2 * pair + 1, :], rhs=ex[:, SC:2 * SC],
                    start=(jc == 0), stop=(jc == N_JC - 1),
                )

            def close_block(pair, ic, outA, outB, inline=False):
                """Normalize one finished attention block.

                Stage 1 (inline): copy PSUM->SBUF so the accumulator banks
                recycle, then 1/Z for both heads. Mid-stream that's the DMA
                reshape bounce ((1,512)->(64,8) so the DVE reciprocal runs at
                8 elems/lane -- a direct (1,512) reciprocal measures 3.3us);
                its ~4us latency hides under the next block. For the FINAL
                block nothing can hide it, so 1/Z = exp(-ln Z) on ScalarE
                instead: Ln and Exp share the natural_log_exp_and_others
                table set (no reload) and ScalarE is idle at the tail.
                Stage 2 (bc broadcast matmul + multiply per head) is deferred
                into the next block's attn slots 4/5 via pending_norm unless
                inline=True."""
                i_sl = slice(ic * SC, (ic + 1) * SC)
                osbA = smpool.tile([65, SC], FP32, name="osbA", tag="osb")
                nc.vector.tensor_copy(osbA, outA)
                osbB = smpool.tile([65, SC], FP32, name="osbB", tag="osb")
                nc.vector.tensor_copy(osbB, outB)

                def rec_bounce(osb):
                    zsp = smpool.tile([64, SC // 64], FP32, name="zsp", tag="zsp")
                    nc.gpsimd.dma_start(out=zsp, in_=osb[64:65, :])
                    rsp = smpool.tile([64, SC // 64], FP32, name="rsp", tag="rsp")
                    nc.vector.reciprocal(rsp, zsp)
                    rec_bf = smpool.tile([65, SC], BF16, name="rec_bf", tag="recbf")
                    nc.gpsimd.dma_start(out=rec_bf[64:65, :], in_=rsp)
                    return rec_bf

                def t2b(recA):
                    bc = prpool.tile([64, SC], FP32, name="bc", tag="ps")
                    nc.tensor.matmul(bc, lhsT=ones65[64:65, :], rhs=recA[64:65, :])
                    nc.vector.tensor_mul(at[pair][0:64, i_sl], osbA[0:64, :], bc)

                def t2c(recB):
                    bc = prpool.tile([64, SC], FP32, name="bc", tag="ps")
                    nc.tensor.matmul(bc, lhsT=ones65[64:65, :], rhs=recB[64:65, :])
                    stg = smpool.tile([64, SC], BF16, name="stg", tag="stg")
                    nc.vector.tensor_mul(stg, osbB[0:64, :], bc)
                    # shift to partitions 64..127 (DVE can't cross lanes)
                    nc.sync.dma_start(out=at[pair][64:128, i_sl], in_=stg)

                if inline:
                    # tail: head B first so its at-shift DMA overlaps t2b
                    recB = rec_bounce(osbB)
                    recA = rec_bounce(osbA)
                    t2c(recB)
                    t2b(recA)
                else:
                    recA = rec_bounce(osbA)
                    recB = rec_bounce(osbB)
                    pending_norm["b"] = lambda: t2b(recA)
                    pending_norm["c"] = lambda: t2c(recB)

            def out_proj_chunk(ic, ec, ss, tail_idx=None):
                """One (128 s, 512 e) chunk of the partial output projection.

                In the tail (tail_idx set) the PSUM->SBUF copies alternate
                between ScalarE (idle after the last exp) and DVE, and the
                stores round-robin over three DMA queues, so the final 8
                chunks drain ~3x faster than a single serialized chain."""
                srow = ic * SC + ss * 128
                po = prpool.tile([128, SC], FP32, name="po", tag="ps")
                for fc in range(2):
                    nc.tensor.matmul(
                        po,
                        lhsT=at[fc][:, srow:srow + 128],
                        rhs=wot[fc][:, ec * SC:(ec + 1) * SC],
                        start=(fc == 0),
                        stop=(fc == 1),
                    )
                stg = spool.tile([128, SC], BF16, name="ostg")
                if tail_idx is not None and tail_idx % 2 == 0:
                    nc.scalar.copy(stg, po)
                else:
                    nc.vector.tensor_copy(stg, po)
                # no gpsimd here: a pending SWDGE store makes the epilogue's
                # GpSimd drain take ~3.6us
                q = nc.sync if tail_idx is None else \
                    (nc.sync, nc.scalar)[tail_idx % 2]
                q.dma_start(
                    out=out.ap()[srow:srow + 128, ec * SC:(ec + 1) * SC],
                    in_=stg,
                )

            # ---- emission order (drives scheduling priority and the
            # per-engine instruction streams; engines execute in order) ----
            #
            # 8 attention blocks (pair, ic). All projection / out-proj work
            # beyond a minimal prefix is dripped into the jc loops at <=1us
            # per slot with deadlines, so the PE stream per jc stays under
            # the ~1.15us exp pace and ScalarE never starves:
            #   block 0 (p0,ic0): vt streaming (all 4 heads; vt[j] by jc=j)
            #                     and k0 halves (s-chunk s by jc=4s)
            #   blocks 1-3:       pair-0 q leftovers, pair-1 q/k
            #   blocks 4-7:       previous ic's out_proj chunks
            K0, Q0, K1, Q1 = (kt[0], wkt, bk_sb, 0), (qt[0], wqt, bq_sb, 0), \
                             (kt[1], wkt, bk_sb, 1), (qt[1], wqt, bq_sb, 1)

            def qk_thunk(args, scn, half):
                dst, w, b, fc = args
                return lambda: qk_half(dst, w, b, scn, fc, half)

            sched = {b: {} for b in range(8)}

            def put(b, jc, thunk):
                sched[b].setdefault(jc, []).append(thunk)

            def q_thunk(args, scn, quarter):
                dst, w, b, fc = args
                return lambda: qk_quarter(dst, w, b, scn, fc, quarter)

            # block 0 is PE-bound regardless (all of v and k0 are needed by
            # its own jc's): v_proj(j) at slot j-1, k0 s-chunk halves ahead
            # of their jc=4s deadlines, q0 sc1 (needed at block 1 jc0) last.
            for j in range(1, N_JC):
                put(0, j - 1, lambda j=j: v_proj(j))
            put(0, 1, qk_thunk(K0, 1, 0)); put(0, 2, qk_thunk(K0, 1, 1))
            put(0, 5, qk_thunk(K0, 2, 0)); put(0, 6, qk_thunk(K0, 2, 1))
            put(0, 9, qk_thunk(K0, 3, 0)); put(0, 10, qk_thunk(K0, 3, 1))
            put(0, 12, qk_thunk(Q0, 1, 0)); put(0, 13, qk_thunk(Q0, 1, 1))
            # blocks 1-4 run at the ScalarE exp pace, whose slack per slot is
            # under half a qk_half: drip the remaining projections as
            # QUARTERS (2 matmuls, ~0.43us), one per slot, keeping slots 0-2
            # (boundary psum refill) and 4-5 (deferred normalize bc+mul)
            # clean. Deadlines: kt1 by slots 63..75, qt1 s-chunk i by slot
            # 16*(4+i)-1.
            quarter_sched = [
                (0, 15, Q0, 2, 0),
                (1, 3, Q0, 2, 1), (1, 6, Q0, 2, 2), (1, 7, Q0, 2, 3),
                (1, 8, Q0, 3, 0), (1, 9, Q0, 3, 1), (1, 10, Q0, 3, 2), (1, 11, Q0, 3, 3),
                (1, 12, K1, 0, 0), (1, 13, K1, 0, 1), (1, 14, K1, 0, 2), (1, 15, K1, 0, 3),
                (2, 3, K1, 1, 0), (2, 6, K1, 1, 1), (2, 7, K1, 1, 2), (2, 8, K1, 1, 3),
                (2, 9, K1, 2, 0), (2, 10, K1, 2, 1), (2, 11, K1, 2, 2), (2, 12, K1, 2, 3),
                (2, 13, K1, 3, 0), (2, 14, K1, 3, 1), (2, 15, K1, 3, 2),
                (3, 3, K1, 3, 3),
                (3, 6, Q1, 0, 0), (3, 7, Q1, 0, 1), (3, 8, Q1, 0, 2), (3, 9, Q1, 0, 3),
                (3, 10, Q1, 1, 0), (3, 11, Q1, 1, 1), (3, 12, Q1, 1, 2), (3, 13, Q1, 1, 3),
                (3, 14, Q1, 2, 0), (3, 15, Q1, 2, 1),
                (4, 3, Q1, 2, 2), (4, 6, Q1, 2, 3),
                (4, 7, Q1, 3, 0), (4, 8, Q1, 3, 1), (4, 9, Q1, 3, 2), (4, 10, Q1, 3, 3),
            ]
            for b, sl, args, scn, q in quarter_sched:
                put(b, sl, q_thunk(args, scn, q))
            # blocks 5-7: drip previous ic's out_proj (8 chunks each),
            # after the slot-4/5 normalize pops that write its `at` input
            for b in range(5, 8):
                ic_prev = b - 5
                idx = 0
                for ec in range(2):
                    for ss in range(SC // 128):
                        put(b, 6 + idx, lambda ic=ic_prev, ec=ec, ss=ss:
                            out_proj_chunk(ic, ec, ss))
                        idx += 1

            # minimal prefix: k0/q0 s-chunk 0 and vt[0] for all heads.
            qk_full(kt[0], wkt, bk_sb, 0, 0)
            qk_full(qt[0], wqt, bq_sb, 0, 0)
            v_proj(0)

            # ---- flat scores-ahead pipeline over 128 (block, jc) slots ----
            # Each slot emits, in order:
            #   1. scores+exp for slot s          (keeps ScalarE one slot
            #                                      ahead -- exp never waits on
            #                                      a scores matmul queued
            #                                      behind drip work)
            #   2. deferred normalize pops        (prev block's bc+mul at
            #                                      attn-slots 1 and 2, when
            #                                      the approx-recip is ready)
            #   3. attn@v for slot s-1
            #   4. drip thunks scheduled at s-1
            # Block close (attn slot 15) runs stage-1 normalize inline and
            # defers stage 2.
            blocks = [(p, i) for p in range(2) for i in range(N_SC)]
            ao_open = {}
            prev = None  # (bi, jc, ex)
            for s in range(129):
                if s < 128:
                    bi, jc = divmod(s, N_JC)
                    pair, ic = blocks[bi]
                    if jc == 0:
                        ao_open[bi] = (
                            aopool.tile([65, SC], FP32, name="outA", tag="ao"),
                            aopool.tile([65, SC], FP32, name="outB", tag="ao"),
                        )
                    ex = scores_exp(pair, ic, jc)
                else:
                    bi = None
                if prev is not None:
                    pbi, pjc, pex = prev
                    ppair, pic = blocks[pbi]
                    if pjc == 4 and "b" in pending_norm:
                        pending_norm.pop("b")()
                    if pjc == 5 and "c" in pending_norm:
                        pending_norm.pop("c")()
                    outA, outB = ao_open[pbi]
                    attn_acc(ppair, pjc, pex, outA, outB)
                    if pjc == N_JC - 1:
                        if pbi == 7:
                            # tail close: preempt the queued out_proj casts
                            # on DVE so the normalize chain starts at once
                            with tc.high_priority(offset=8_000_000):
                                close_block(ppair, pic, outA, outB,
                                            inline=True)
                        else:
                            close_block(ppair, pic, outA, outB)
                        del ao_open[pbi]
                    for thunk in sched[pbi].get(pjc, []):
                        thunk()
                prev = (bi, jc, ex) if s < 128 else None
            # final ic's output projection (tail)
            idx = 0
            for ec in range(2):
                for ss in range(SC // 128):
                    out_proj_chunk(N_SC - 1, ec, ss, tail_idx=idx)
                    idx += 1

    nc.compile()
    return nc


_NC_CACHE = None


def _get_nc():
    global _NC_CACHE
    if _NC_CACHE is None:
        _NC_CACHE = build_kernel()
    return _NC_CACHE


def make_in_maps(inputs):
    x = np.asarray(inputs["x"], dtype=np.float32)
    mask = np.asarray(inputs["mask"])
    Wq = np.asarray(inputs["Wq"], dtype=np.float32)
    bq = np.asarray(inputs["bq"], dtype=np.float32)
    Wk = np.asarray(inputs["Wk"], dtype=np.float32)
    bk = np.asarray(inputs["bk"], dtype=np.float32)
    Wv = np.asarray(inputs["Wv"], dtype=np.float32)
    bv = np.asarray(inputs["bv"], dtype=np.float32)
    Wo = np.asarray(inputs["Wo"], dtype=np.float32)

    bf = ml_dtypes.bfloat16

    def pack_dxf(wT):  # (1024, FL) -> (128, 8*FL): d-chunks side by side
        return np.ascontiguousarray(
            wT.reshape(DCH, 128, FL).transpose(1, 0, 2).reshape(128, DCH * FL)
        )

    def pack_fxe(woT):  # (256, D) -> (128, 2*D): f-chunks side by side
        return np.ascontiguousarray(
            woT.reshape(2, 128, D).transpose(1, 0, 2).reshape(128, 2 * D)
        )

    def pack_xP(xT):  # (1024, 2048) -> (128, 4*8*512) s-chunk-major
        # xP[p, sc*4096 + dc*512 + c] = xT[dc*128+p, sc*512+c]
        v = xT.reshape(DCH, 128, N_SC, SC)        # (dc, p, sc, c)
        return np.ascontiguousarray(
            v.transpose(1, 2, 0, 3).reshape(128, N_SC * DCH * SC)
        )

    in_maps = []
    for c in range(NCORES):
        b = c // GROUPS
        g = c % GROUPS
        fs, fe = g * FL, (g + 1) * FL
        in_maps.append({
            "xP": pack_xP(np.ascontiguousarray(x[b].T).astype(bf)),
            "wq": pack_dxf(Wq[fs:fe, :].T.astype(bf)),
            "wk": pack_dxf(Wk[fs:fe, :].T.astype(bf)),
            "wv": pack_dxf(Wv[fs:fe, :].T.astype(bf)),
            "wo": pack_fxe(Wo[:, fs:fe].T.astype(bf)),
            "bqc": np.ascontiguousarray(bq[fs:fe].reshape(2, 128).T),
            "bkc": np.ascontiguousarray(bk[fs:fe].reshape(2, 128).T),
            "bvb": np.tile(bv[fs:fe], (128, 1)).astype(np.float32),
            "mb": np.ascontiguousarray(
                np.where(mask[b] == 0, np.float32(-1e9), np.float32(0.0))
                .astype(np.float32).reshape(N_JC, 128).T
            ),
        })
    return in_maps


def kernel(x, mask, Wq, bq, Wk, bk, Wv, bv, Wo, bo):
    bo = np.asarray(bo, dtype=np.float32)
    nc = _get_nc()
    in_maps = make_in_maps(dict(x=x, mask=mask, Wq=Wq, bq=bq, Wk=Wk, bk=bk,
                                Wv=Wv, bv=bv, Wo=Wo, bo=bo))
    res = run_bass_kernel_spmd(nc, in_maps, core_ids=list(range(NCORES)))
    parts = [np.asarray(r["out"], dtype=np.float32) for r in res.results]
    full = np.empty((B, S, D), dtype=np.float32)
    for b in range(B):
        acc = parts[b * GROUPS].copy()
        for g in range(1, GROUPS):
            acc += parts[b * GROUPS + g]
        full[b] = acc + bo[None, :]
    return full
